# revision 6
# baseline (speedup 1.0000x reference)
"""2-layer GCN (GCNConv without normalization) as a Bass/Trainium2 SPMD kernel on 8 NeuronCores.

Strategy (graph/data parallel, node sharding) — bf16 data path:
  - Nodes are sorted by in-degree and dealt round-robin to the 8 cores, so
    every core owns ~E/8 edges and windows of 128 nodes have near-uniform
    max in-degree (~1% gather padding).
  - Layer algebra: segment_sum commutes with the linear maps, so both layers
    aggregate 16-wide tables:
        L1: h1 = x @ W1 (16 wide); agg1 = scatter_add(w * h1[src]);
            a1 = relu(agg1 + b1)
        L2: agg2 = scatter_add(w * a1[src]); out = log_softmax(agg2 @ W2 + b2)
  - Tables are bf16, stored as 4-node packs in 256B-strided DRAM rows
    (128B payload + 128B pad).  Each destination slot fetches its source's
    pack with one dma_gather descriptor (int16 pack ids < 25088).  The
    gather cost on TRN2 is latency-bound at ~2ns/descriptor regardless of
    element size, so 128B bf16 elements cost the same as the f32 256B packs
    but halve every downstream cost (DVE select/reduce in 2x mode, AllGather
    volume, SBUF staging).
  - The per-edge weight is a 4-wide bf16 one-hot (w at the source's position
    in its pack): one in-place DVE multiply + one strided tensor_reduce per
    128-node window performs pack selection and segment sum together.
  - Table row of node (core, p, w) is w*128+p (w-major) so bf16 4-packs are
    partition-quads within one window column and the padded shard write is a
    single strided DMA.

The harness calls kernel(**inputs) with full inputs; sharding happens here.
"""

import os
import sys
import time

import numpy as np
import ml_dtypes

sys.path.insert(0, "/opt/trn_rl_repo")

NCORES = 8
P = 128
NQ = 4           # SWDGE queues to rotate dma_gather over
CS_MAX = 63      # slot columns per chunk

# stash for test harness introspection (exec time, etc.)
LAST = {}

BF16 = ml_dtypes.bfloat16


def _host_prep(x, edge_index, edge_weight):
    """Build the sharded/padded data layout. Returns (cfg, per_core_arrays)."""
    N, Fin = x.shape
    E = edge_index.shape[1]
    src = np.ascontiguousarray(edge_index[0]).astype(np.int64)
    dst = np.ascontiguousarray(edge_index[1]).astype(np.int64)

    deg = np.bincount(dst, minlength=N)
    order = np.argsort(-deg, kind="stable")  # node ids, highest in-degree first
    rank_of = np.empty(N, np.int64)
    rank_of[order] = np.arange(N)
    core_of = rank_of % NCORES
    lrank = rank_of // NCORES

    NLOC = -(-N // NCORES)          # nodes per core
    W = -(-NLOC // P)               # windows per core
    NPAD = W * P
    W2 = -(-W // 4) * 4             # table windows padded to a multiple of 4
    NPK = W2 * P // 4               # 4-node packs per core (w-quads per p)
    w_of = lrank // P
    p_of = lrank % P
    # gather-table row: p-major with W2 stride so 4-packs are w-quads of one p
    tbl = (core_of * (W2 * P) + p_of * W2 + w_of).astype(np.int64)

    # per-window max degree K[w]
    K = np.zeros(W, np.int64)
    for w in range(W):
        lo = w * P * NCORES
        hi = min((w + 1) * P * NCORES, N)
        K[w] = deg[order[lo:hi]].max() if hi > lo else 0
    K = np.maximum(K, 1)
    OFF = np.concatenate([[0], np.cumsum(K)]).astype(np.int64)
    S_total = int(OFF[-1])

    # chunk windows so each chunk's gather fits one dma_gather instruction
    chunks = []
    w0, s0 = 0, 0
    for w in range(W):
        if s0 + K[w] > CS_MAX and w > w0:
            chunks.append((w0, w, int(OFF[w0]), int(s0)))
            w0, s0 = w, 0
        s0 += K[w]
    chunks.append((w0, W, int(OFF[w0]), int(s0)))

    # edge slots: j-th edge of node d -> (core_of[d], p_of[d], OFF[w_of[d]] + j)
    e_sort = np.argsort(dst, kind="stable")
    ds = dst[e_sort]
    ss = src[e_sort]
    ws = np.ascontiguousarray(edge_weight)[e_sort].astype(np.float32)
    starts = np.concatenate([[0], np.cumsum(deg)])
    j = np.arange(E) - starts[ds]
    col = OFF[w_of[ds]] + j

    # 4-node pack id + position-in-pack one-hot weights (bf16)
    pack_arr = np.zeros((NCORES, P, S_total), np.int16)
    pack_arr[core_of[ds], p_of[ds], col] = (tbl[ss] // 4).astype(np.int16)
    w4_arr = np.zeros((NCORES, P, S_total, 4), np.float32)
    w4_arr[core_of[ds], p_of[ds], col, tbl[ss] % 4] = ws

    # idx16: position g = col*128 + p reads int16 idx at [band*16 + g%16,
    # g//16]; replicate into all 8 bands.
    NCOLS = S_total * 8
    idx16 = np.empty((NCORES, P, NCOLS), np.int16)
    for k in range(NCORES):
        g_order = pack_arr[k].T.reshape(-1)            # g = col*128 + p
        band = g_order.reshape(NCOLS, 16).T            # [16, NCOLS]
        idx16[k] = np.tile(band, (8, 1))
    w4d = np.ascontiguousarray(
        w4_arr.reshape(NCORES, P, S_total * 4)).astype(BF16)

    # x^T shards: core k, physical slot (p, w) holds node with lrank w*128+p?
    # No: keep lrank l at physical (p, w) = (l % P, l // P) like before; only
    # the TABLE row label changed to w*128+p.
    node_of = order[: NLOC * NCORES].reshape(NLOC, NCORES)
    xts = np.zeros((NCORES, Fin, NPAD), np.float32)
    for k in range(NCORES):
        xts[k, :, :NLOC] = x[node_of[:, k]].T

    # physical placement: lrank l -> (p_of, w_of) = (l % P, l // P); the
    # h1_sb column of l is w_of*H..; table row = w_of*128 + p_of.
    l = np.arange(NLOC)
    out_rows = (l % P) * W + l // P  # row in kernel output for local slot l

    cfg = dict(
        N=N, Fin=Fin, E=E, NLOC=NLOC, W=W, W2=W2, NPAD=NPAD, NPK=NPK,
        K=K.tolist(), OFF=OFF.tolist(), S_total=S_total, chunks=chunks,
    )
    return cfg, idx16, w4d, xts, node_of, out_rows


def _dma_gather_raw(g, out_ap, in_ap, idxs_ap, num_idxs, elem_size,
                    stride_elems, single_packet=False, queue_num=0):
    """dma_gather with sub-256B elem_size (ucode supports it for
    non-transpose; the bass-level %256 assert is transpose-only)."""
    import concourse.mybir as mybir
    _in = g.lower_ap_dma(in_ap, for_custom_bir_dma=True)
    sz = mybir.dt.size(in_ap.dtype)
    return g.add_instruction(mybir.InstDMAGatherAnt(
        name=g.bass.get_next_instruction_name(),
        ins=[*_in, g.lower_ap(idxs_ap),
             g.lower_val_access(g.to_reg(num_idxs))],
        outs=[g.lower_ap(out_ap)],
        transpose=False, num_idxs=num_idxs, elem_size=elem_size,
        stride_bytes_256=(stride_elems * sz) // 256, gen_mode=0,
        single_packet=single_packet, queue_num=queue_num,
        sbuf_tokens_per_rank=0, sbuf_free_dim_per_rank=0,
        sbuf_free_dim_pad_per_rank=0, sbuf_byte_offset=0))


def _build(nc, cfg, H, C):
    """Emit the Bass/Tile program (identical on all cores)."""
    import concourse.mybir as mybir
    import concourse.tile as tile
    from concourse import bass
    from concourse.masks import make_identity

    Fin, W, NPAD, S_total = cfg["Fin"], cfg["W"], cfg["NPAD"], cfg["S_total"]
    NPK, TW2 = cfg["NPK"], cfg["W2"]
    K, OFF, chunks = cfg["K"], cfg["OFF"], cfg["chunks"]
    KB = Fin // P  # K-blocks for the x @ W1 matmul
    f32 = mybir.dt.float32
    bf16 = mybir.dt.bfloat16

    xT = nc.dram_tensor("xT", [Fin, NPAD], f32, kind="ExternalInput")
    W1 = nc.dram_tensor("W1", [Fin, H], f32, kind="ExternalInput")
    W2 = nc.dram_tensor("W2", [H, C], bf16, kind="ExternalInput")
    b1b = nc.dram_tensor("b1b", [P, H], bf16, kind="ExternalInput")
    b2b = nc.dram_tensor("b2b", [P, C], f32, kind="ExternalInput")
    idx = nc.dram_tensor("idx", [P, S_total * 8], mybir.dt.int16, kind="ExternalInput")
    w4 = nc.dram_tensor("w4", [P, S_total * 4], bf16, kind="ExternalInput")
    out = nc.dram_tensor("out", [NPAD, C], f32, kind="ExternalOutput")

    # 4-node packs, 256B-strided rows: [NPK, 128] bf16, payload in cols 0:64
    h1_shard = nc.dram_tensor("h1_shard", [NPK, P], bf16)
    h1_full = nc.dram_tensor("h1_full", [NCORES * NPK, P], bf16, addr_space="Shared")
    a1_shard = nc.dram_tensor("a1_shard", [NPK, P], bf16)
    a1_full = nc.dram_tensor("a1_full", [NCORES * NPK, P], bf16, addr_space="Shared")

    XTW = 4  # windows per xT load chunk

    CS_ALLOC = max(c[3] for c in chunks)

    def bcast_mid(ap, n):
        """[P, F] -> [P, n, F] with a step-0 middle dim."""
        return bass.AP(ap.tensor, ap.offset, [list(ap.ap[0]), [0, n], list(ap.ap[1])])

    WQ = W // 4          # full w-quads (24)
    WR = W - WQ * 4      # leftover windows (2)
    PKP = TW2 // 4       # packs per partition (25)

    def write_shard(dram_t, sb):
        """sb [P, W*H] bf16 -> packed [NPK, 128] rows; pack (p, w//4),
        in-pack slot w%4. Plus zero-fill of the 2 pad slots per last quad."""
        # full quads: contiguous 64-elem blocks on both sides
        nc.sync.dma_start(
            out=bass.AP(dram_t, 0,
                        [[PKP * P, P], [P, WQ], [1, 4 * H]]),
            in_=bass.AP(sb.tensor, sb.offset,
                        [list(sb.ap[0]), [4 * H, WQ], [1, 4 * H]]),
        )
        # leftover windows (w = WQ*4 .. W-1) into the last quad's first slots
        nc.sync.dma_start(
            out=bass.AP(dram_t, WQ * P,
                        [[PKP * P, P], [1, WR * H]]),
            in_=bass.AP(sb.tensor, sb.offset + WQ * 4 * H,
                        [list(sb.ap[0]), [1, WR * H]]),
        )
        # zero the 2 pad slots of the last quad (avoid NaN garbage * 0)
        nc.sync.dma_start(
            out=bass.AP(dram_t, WQ * P + WR * H,
                        [[PKP * P, P], [1, (4 - WR) * H]]),
            in_=zero_sb[:, 0:(4 - WR) * H],
        )

    GBUFS = 8
    with tile.TileContext(nc) as tc:
        with (
            tc.tile_pool(name="const", bufs=1) as constp,
            tc.tile_pool(name="persist", bufs=1) as persist,
            tc.tile_pool(name="xt", bufs=2) as xtp,
            tc.tile_pool(name="gath", bufs=GBUFS) as gathp,
            tc.tile_pool(name="idxm", bufs=8) as idxmp,
            tc.tile_pool(name="w4m", bufs=8) as w4mp,
            tc.tile_pool(name="gt", bufs=2) as gtp,
            tc.tile_pool(name="ps_h", bufs=2, space="PSUM") as ps_h,
            tc.tile_pool(name="ps_t", bufs=2, space="PSUM") as ps_t,
            tc.tile_pool(name="ps_o", bufs=2, space="PSUM") as ps_o,
        ):
            # constants
            w1_sb = constp.tile([P, KB * H], f32, tag="w1")
            for kb in range(KB):
                nc.sync.dma_start(out=w1_sb[:, kb * H:(kb + 1) * H],
                                  in_=W1[kb * P:(kb + 1) * P, :])
            w2_sb = constp.tile([H, C], bf16, tag="w2")
            nc.sync.dma_start(out=w2_sb[:, :], in_=W2[:, :])
            b1_sb = constp.tile([P, H], bf16, tag="b1")
            nc.sync.dma_start(out=b1_sb[:, :], in_=b1b[:, :])
            b2_sb = constp.tile([P, C], f32, tag="b2")
            nc.sync.dma_start(out=b2_sb[:, :], in_=b2b[:, :])
            ident = constp.tile([P, P], bf16, tag="ident")
            make_identity(nc, ident[:])
            zero_sb = constp.tile([P, 2 * H], bf16, tag="zero")
            nc.vector.memset(zero_sb[:, :], 0.0)

            h1_sb = persist.tile([P, W * H], bf16, tag="h1")
            agg_sb = persist.tile([P, W * H], bf16, tag="agg")
            o_sb = persist.tile([P, W * C], f32, tag="o")
            e_sb = persist.tile([P, W * C], f32, tag="e")
            red_sb = persist.tile([P, 2 * W], f32, tag="red")

            # ---- Phase 1: h1 = x @ W1, per 128-node window ----
            for wc in range(0, W, XTW):
                nw = min(XTW, W - wc)
                xt_sb = xtp.tile([P, KB, XTW * P], f32, tag="xt")
                for kb in range(KB):
                    nc.sync.dma_start(
                        out=xt_sb[:, kb, : nw * P],
                        in_=xT[kb * P:(kb + 1) * P, wc * P:(wc + nw) * P],
                    )
                for w in range(wc, wc + nw):
                    ph = ps_h.tile([P, H], f32, tag="ph")
                    for kb in range(KB):
                        nc.tensor.matmul(
                            out=ph[:, :],
                            lhsT=xt_sb[:, kb, (w - wc) * P:(w - wc + 1) * P],
                            rhs=w1_sb[:, kb * H:(kb + 1) * H],
                            start=(kb == 0),
                            stop=(kb == KB - 1),
                        )
                    nc.scalar.copy(out=h1_sb[:, w * H:(w + 1) * H], in_=ph[:, :])

            # write packed shard and AllGather
            write_shard(h1_shard, h1_sb[:, :])
            nc.gpsimd.collective_compute(
                "AllGather",
                mybir.AluOpType.bypass,
                replica_groups=[list(range(NCORES))],
                ins=[h1_shard[:, :]],
                outs=[h1_full[:, :]],
            )

            # ---- Phases 2/4: packed edge gather + one-hot weight + reduce ----
            def edge_layer(table, dst_sb, li):
                table4 = table[:, 0:4 * H]  # [rows, 64] bf16, 256B stride
                for ci, (w0, w1, off0, S_c) in enumerate(chunks):
                    idx_t = idxmp.tile([P, CS_ALLOC * 8], mybir.dt.int16, tag="idxc")
                    nc.sync.dma_start(out=idx_t[:, : S_c * 8],
                                      in_=idx[:, off0 * 8:(off0 + S_c) * 8])
                    w4_t = w4mp.tile([P, CS_ALLOC * 4], bf16, tag="w4c")
                    nc.sync.dma_start(out=w4_t[:, : S_c * 4],
                                      in_=w4[:, off0 * 4:(off0 + S_c) * 4])
                    ga = gathp.tile([P, CS_ALLOC * 64], bf16, tag="ga")
                    ca = S_c // 2
                    for hi, (c0, c1) in enumerate(((0, ca), (ca, S_c))):
                        _dma_gather_raw(
                            nc.gpsimd,
                            ga[:, c0 * 64: c1 * 64].rearrange(
                                "p (m e) -> p m e", e=64),
                            table4,
                            idx_t[:, c0 * 8: c1 * 8],
                            (c1 - c0) * P, 64, P,
                            queue_num=(2 * (li * len(chunks) + ci) + hi) % NQ,
                            single_packet=False,
                        )
                    # msg *= w4 (pack-position one-hot weight, broadcast over H)
                    ga3 = ga[:, : S_c * 64].rearrange("p (m h) -> p m h", h=H)
                    nc.vector.tensor_tensor(
                        out=ga3,
                        in0=ga3,
                        in1=w4_t[:, : S_c * 4].to_broadcast([P, S_c * 4, H]),
                        op=mybir.AluOpType.mult,
                    )
                    with nc.allow_low_precision(
                            reason="bf16 segment sums are within the 2e-2 "
                                   "tolerance (DVE accumulates fp32 internally)"):
                        for w in range(w0, w1):
                            o = (OFF[w] - off0) * 64
                            nc.vector.tensor_reduce(
                                out=dst_sb[:, w * H:(w + 1) * H],
                                in_=ga[:, o: o + K[w] * 64].rearrange(
                                    "p (s h) -> p h s", h=H),
                                axis=mybir.AxisListType.X,
                                op=mybir.AluOpType.add,
                            )

            edge_layer(h1_full, agg_sb, 0)

            # ---- Phase 3: a1 = relu(agg1 + b1); share and AllGather ----
            agg3 = agg_sb[:, :].rearrange("p (w h) -> p w h", h=H)
            nc.vector.tensor_tensor(
                out=agg3, in0=agg3, in1=bcast_mid(b1_sb[:, :], W),
                op=mybir.AluOpType.add,
            )
            nc.vector.tensor_scalar_max(out=agg_sb[:, :], in0=agg_sb[:, :], scalar1=0.0)
            write_shard(a1_shard, agg_sb[:, :])
            nc.gpsimd.collective_compute(
                "AllGather",
                mybir.AluOpType.bypass,
                replica_groups=[list(range(NCORES))],
                ins=[a1_shard[:, :]],
                outs=[a1_full[:, :]],
            )

            edge_layer(a1_full, h1_sb, 1)  # reuse h1_sb as G (L2 aggregate)

            # ---- Phase 5: out = log_softmax(G @ W2 + b2) ----
            for w in range(W):
                pt = ps_t.tile([H, P], bf16, tag="pt")
                nc.tensor.transpose(
                    out=pt[:, :], in_=h1_sb[:, w * H:(w + 1) * H], identity=ident[:]
                )
                gt_sb = gtp.tile([H, P], bf16, tag="gt")
                nc.scalar.copy(out=gt_sb[:, :], in_=pt[:, :])
                po = ps_o.tile([P, C], f32, tag="po")
                nc.tensor.matmul(
                    out=po[:, :], lhsT=gt_sb[:, :], rhs=w2_sb[:, :],
                    start=True, stop=True,
                )
                nc.scalar.copy(out=o_sb[:, w * C:(w + 1) * C], in_=po[:, :])

            o3 = o_sb[:, :].rearrange("p (w c) -> p w c", c=C)
            nc.vector.tensor_tensor(
                out=o3, in0=o3,
                in1=bcast_mid(b2_sb[:, :], W),
                op=mybir.AluOpType.add,
            )
            rmax = red_sb[:, 0:W]
            rsum = red_sb[:, W:2 * W]
            nc.vector.tensor_reduce(out=rmax, in_=o3, axis=mybir.AxisListType.X,
                                    op=mybir.AluOpType.max)
            nc.vector.tensor_tensor(out=o3, in0=o3,
                                    in1=rmax.to_broadcast([P, W, C]),
                                    op=mybir.AluOpType.subtract)
            nc.scalar.activation(out=e_sb[:, :], in_=o_sb[:, :],
                                 func=mybir.ActivationFunctionType.Exp)
            nc.vector.tensor_reduce(out=rsum,
                                    in_=e_sb[:, :].rearrange("p (w c) -> p w c", c=C),
                                    axis=mybir.AxisListType.X,
                                    op=mybir.AluOpType.add)
            nc.scalar.activation(out=rsum, in_=rsum,
                                 func=mybir.ActivationFunctionType.Ln)
            nc.vector.tensor_tensor(out=o3, in0=o3,
                                    in1=rsum.to_broadcast([P, W, C]),
                                    op=mybir.AluOpType.subtract)
            nc.sync.dma_start(
                out=out[:, :].rearrange("(p w) c -> p (w c)", p=P),
                in_=o_sb[:, :],
            )
    return None


def kernel(x, edge_index, edge_weight, W1, b1, W2, b2):
    import concourse.bacc as bacc
    from concourse.bass_utils import run_bass_kernel_spmd

    x = np.asarray(x, dtype=np.float32)
    W1 = np.asarray(W1, dtype=np.float32)
    b1 = np.asarray(b1, dtype=np.float32)
    W2 = np.asarray(W2, dtype=np.float32)
    b2 = np.asarray(b2, dtype=np.float32)
    edge_weight = np.asarray(edge_weight, dtype=np.float32)
    edge_index = np.asarray(edge_index)

    N = x.shape[0]
    H = W1.shape[1]
    C = W2.shape[1]

    t0 = time.time()
    cfg, idx16, w4d, xts, node_of, out_rows = _host_prep(x, edge_index, edge_weight)
    LAST["prep_s"] = time.time() - t0

    t0 = time.time()
    nc = bacc.Bacc("TRN2", target_bir_lowering=False, debug=False,
                   num_devices=NCORES, num_swdge_queues=NQ)
    _build(nc, cfg, H, C)
    nc.compile()
    LAST["build_s"] = time.time() - t0

    b1b = np.broadcast_to(b1, (P, H)).astype(BF16).copy()
    b2b = np.broadcast_to(b2, (P, C)).copy()
    in_maps = [
        {
            "xT": xts[k],
            "W1": W1, "W2": W2.astype(BF16), "b1b": b1b, "b2b": b2b,
            "idx": idx16[k], "w4": w4d[k],
        }
        for k in range(NCORES)
    ]

    t0 = time.time()
    res = run_bass_kernel_spmd(
        nc, in_maps, core_ids=list(range(NCORES)),
        trace=bool(int(os.environ.get("GCN_TRACE", "0"))),
    )
    LAST["run_s"] = time.time() - t0
    LAST["results"] = res
    LAST["cfg"] = cfg

    outf = np.empty((N, C), np.float32)
    for k in range(NCORES):
        outf[node_of[:, k]] = res.results[k]["out"][out_rows]
    return outf
